# revision 16
# baseline (speedup 1.0000x reference)
"""DecoderTreeLSTMCell Trainium2 Bass kernel.

Strategy: data-parallel over nodes on 8 cores (4096 nodes/core). Host groups
each core's nodes into a column layout [mask0 region | mask1 region], each
region ordered by pos (10 blocks), each block sub-ordered [d0 | d1 | d2]
(d1/d2 = nodes needing the extras add, kept at the block tail). Capacities are
compile-time (max over cores, round-robin deal => ~zero padding).

Everything streams as fp16: ONE input tensor AIN [128, Lin] fp16 with regions
[child_h(L) | child_c(L) | extras(E)], ONE output OUT [128, Lout] fp16 with
regions [h_new(M0) | c_new(M0) | c_red(M1)] - a single DMA each way per pass
(HWDGE descriptor-gen is ~0.6us of serialized queue time per dma_start).

The per-pos f bias b_f is folded into the u matmul via a K=10 one-hot matmul
(stationary = [10,128] b_f rows, moving = [10,C] pos-indicator, PSUM
accumulate). That makes every activation pos-independent, so sigmoids/tanhs
run as one ACT instruction per full 512-col PSUM window instead of per
(pos,mask) chunk - the ACT engine (1 elem/cycle/lane, the bottleneck at
~12.3K cols/core) spends its time on elements, not instruction overhead.
i/o/u biases ride the ACT bias AP (free). All DVE elementwise work is fp16
(2x mode). Matmuls are fp16 (1 cycle/row vs fp32r's 4x penalty below 256
cols).

Per mask0 window: u/i/o/t matmuls per pos piece + one bias matmul, then
f=sig(u), si=sig(i+bi), tu=tanh(t+bu), c_red=f*c, c_new=si*tu+c_red,
so=sig(o+bo), h=so*tanh(c_new). Mask1 windows: u matmuls + f=sig(u),
c_red=f*c only. h rows with mask=1 equal h_prev exactly and are filled
host-side during unshard (data routing only).
"""
import numpy as np

import concourse.bacc as bacc
import concourse.mybir as mybir
from concourse.tile import TileContext
from concourse.bass_utils import run_bass_kernel_spmd

N = 32768
H = 128
N_POS = 10
NC = 8
SH = N // NC

F32 = mybir.dt.float32
F16 = mybir.dt.float16
Sig = mybir.ActivationFunctionType.Sigmoid
Tanh = mybir.ActivationFunctionType.Tanh

WIN = 512    # PSUM window: 1 bank
BANK = 512   # matmul out must stay within one bank

LAST = {}


def _roundup(x, m):
    return ((x + m - 1) // m) * m


def _plan(pos, depth, mask):
    """Column layout + window/piece schedule.

    Returns dict with:
      L, M0, Lin, Lout, Etot
      blocks: {(p, m): (off, c0, w)}  off absolute in L, c0 = no-extras count,
              w = extras tail width; block len = c0 + w
      e_off: {(p, m): offset of its extras columns within the E region}
      windows: list of (m, lo, hi, pieces) absolute [lo,hi) in L,
               pieces = [(p, plo, phi)] absolute
      slot_idx [NC, L], ain_slot/ain_kind [Lin], out_slot/out_kind [Lout]
    """
    dcl = np.where(depth == 1, 1, np.where(depth == 2, 2, 0))
    idx = {}
    counts = np.zeros((NC, N_POS, 2, 3), np.int64)
    for p in range(N_POS):
        for m in range(2):
            for k in range(3):
                gg = np.nonzero((pos == p) & (mask == m) & (dcl == k))[0]
                for c in range(NC):
                    ii = gg[c::NC]
                    idx[(c, p, m, k)] = ii
                    counts[c, p, m, k] = len(ii)

    caps = np.zeros((N_POS, 2, 3), np.int64)
    for p in range(N_POS):
        for m in range(2):
            for k in range(3):
                caps[p, m, k] = _roundup(int(counts[:, p, m, k].max()), 4)

    blocks = {}
    sub_off = {}
    e_off = {}
    off = 0
    eo = 0
    region_lo = [0, 0]
    for m in range(2):
        region_lo[m] = off
        for p in range(N_POS):
            c0 = int(caps[p, m, 0])
            w = int(caps[p, m, 1] + caps[p, m, 2])
            # pad each block to 256 so blocks sit 2-per-PSUM-bank: no piece
            # ever crosses a 512 bank boundary -> minimum matmul count
            c0 += _roundup(c0 + w, 256) - (c0 + w)
            blocks[(p, m)] = (off, c0, w)
            # block layout [d0 | pad | d1 | d2]: extras tail starts at c0
            sub_off[(p, m, 0)] = off
            sub_off[(p, m, 1)] = off + c0
            sub_off[(p, m, 2)] = off + c0 + int(caps[p, m, 1])
            e_off[(p, m)] = eo
            eo += w
            off += c0 + w
    M0 = region_lo[1]
    L = off
    Etot = eo
    Lin = 2 * L + Etot
    Lout = 2 * M0 + (L - M0)

    windows = []
    for m in range(2):
        r0 = region_lo[m]
        r1 = M0 if m == 0 else L
        start = r0
        while start < r1:
            end = min(start + WIN, r1)
            pieces = []
            for p in range(N_POS):
                boff, c0, w = blocks[(p, m)]
                blen = c0 + w
                lo2, hi2 = max(boff, start), min(boff + blen, end)
                if lo2 < hi2:
                    # split at PSUM bank boundaries (window-relative 512s)
                    s = lo2
                    while s < hi2:
                        nb = start + _roundup(s - start + 1, BANK)
                        e = min(hi2, nb)
                        pieces.append((p, s, e))
                        s = e
            windows.append((m, start, end, pieces))
            start = end
    # interleave mask0/mask1 windows for a smoother engine mix
    w0 = [w for w in windows if w[0] == 0]
    w1 = [w for w in windows if w[0] == 1]
    inter = []
    for i in range(max(len(w0), len(w1))):
        if i < len(w0):
            inter.append(w0[i])
        if i < len(w1):
            inter.append(w1[i])
    windows = inter

    slot_idx = np.full((NC, L), -1, np.int64)
    for c in range(NC):
        for p in range(N_POS):
            for m in range(2):
                for k in range(3):
                    ii = idx[(c, p, m, k)]
                    o = sub_off[(p, m, k)]
                    slot_idx[c, o:o + len(ii)] = ii

    ain_slot = np.full(Lin, -1, np.int64)
    ain_kind = np.zeros(Lin, np.int64)
    ain_slot[0:L] = np.arange(L)
    ain_kind[0:L] = 0
    ain_slot[L:2 * L] = np.arange(L)
    ain_kind[L:2 * L] = 1
    for m in range(2):
        for p in range(N_POS):
            boff, c0, w = blocks[(p, m)]
            a = 2 * L + e_off[(p, m)]
            ain_slot[a:a + w] = np.arange(boff + c0, boff + c0 + w)
            ain_kind[a:a + w] = 2

    out_slot = np.full(Lout, -1, np.int64)
    out_kind = np.zeros(Lout, np.int64)
    out_slot[0:M0] = np.arange(M0)
    out_kind[0:M0] = 0
    out_slot[M0:2 * M0] = np.arange(M0)
    out_kind[M0:2 * M0] = 1
    out_slot[2 * M0:] = np.arange(M0, L)
    out_kind[2 * M0:] = 2

    return dict(L=L, M0=M0, Lin=Lin, Lout=Lout, Etot=Etot, blocks=blocks,
                e_off=e_off, windows=windows, slot_idx=slot_idx,
                ain_slot=ain_slot, ain_kind=ain_kind,
                out_slot=out_slot, out_kind=out_kind)


def _build(plan, reps=1, bodies=1):
    L, M0, Lin, Lout = plan["L"], plan["M0"], plan["Lin"], plan["Lout"]
    blocks, e_off, windows = plan["blocks"], plan["e_off"], plan["windows"]
    WCOLS = N_POS * 4 * H          # per-pos weight blocks
    BF_OFF = WCOLS                 # b_f: row 0, per pos at BF_OFF + p*H

    nc = bacc.Bacc("TRN2", target_bir_lowering=False)
    AIN = nc.dram_tensor("AIN", [H, Lin], F16, kind="ExternalInput")
    W = nc.dram_tensor("W", [H, WCOLS + N_POS * H], F16,
                       kind="ExternalInput")
    OH = nc.dram_tensor("OH", [1, WIN], F16, kind="ExternalInput")
    BIAS = nc.dram_tensor("BIAS", [H, 3 + N_POS], F32,
                          kind="ExternalInput")
    OUT = nc.dram_tensor("OUT", [H, Lout], F16, kind="ExternalOutput")

    with TileContext(nc) as tc:
        with (
            tc.tile_pool(name="const", bufs=1) as cpool,
            tc.tile_pool(name="io", bufs=3) as io,
            tc.tile_pool(name="ot", bufs=2) as ot,
            tc.tile_pool(name="wk", bufs=3) as wk,
            tc.tile_pool(name="ps_u", bufs=2, space="PSUM") as ps_u,
            tc.tile_pool(name="ps_i", bufs=2, space="PSUM") as ps_i,
            tc.tile_pool(name="ps_o", bufs=2, space="PSUM") as ps_o,
            tc.tile_pool(name="ps_t", bufs=2, space="PSUM") as ps_t,
        ):
            w_sb = cpool.tile([H, WCOLS + N_POS * H], F16, tag="w")
            nc.sync.dma_start(out=w_sb[:, :], in_=W[:, :])
            oh_sb = cpool.tile([1, WIN], F16, tag="oh")
            nc.sync.dma_start(out=oh_sb[:, :], in_=OH[:, :])
            bias_sb = cpool.tile([H, 3 + N_POS], F32, tag="bias")
            nc.sync.dma_start(out=bias_sb[:, :], in_=BIAS[:, :])
            def body(j, store_prev=True):
                ain = io.tile([H, Lin], F16, tag="ain")
                nc.sync.dma_start(out=ain[:, :], in_=AIN[:, :])
                out = ot.tile([H, Lout], F16, tag="out")

                for m in range(2):
                    for p in range(N_POS):
                        boff, c0, w = blocks[(p, m)]
                        if w:
                            t0 = boff + c0
                            a = 2 * L + e_off[(p, m)]
                            nc.vector.tensor_add(
                                ain[:, t0:t0 + w], ain[:, t0:t0 + w],
                                ain[:, a:a + w])

                for (m, lo, hi, pieces) in windows:
                    C = hi - lo
                    p_u = ps_u.tile([H, C], F32, tag="u")
                    for (p, plo, phi) in pieces:
                        ap = p_u[:, plo - lo:phi - lo]
                        nc.tensor.matmul(
                            ap, w_sb[:, p * 4 * H:p * 4 * H + H],
                            ain[:, plo:phi], start=True, stop=False)
                        nc.tensor.matmul(
                            ap, w_sb[0:1, BF_OFF + p * H:BF_OFF + (p + 1) * H],
                            oh_sb[0:1, 0:phi - plo], start=False, stop=True)
                    f_sb = wk.tile([H, C], F16, tag="f")
                    nc.scalar.activation(f_sb[:, :], p_u[:, :], Sig)

                    cL = L + lo
                    if m == 0:
                        cr_sb = wk.tile([H, C], F16, tag="cr")
                        nc.vector.tensor_mul(cr_sb[:, :], f_sb[:, :],
                                             ain[:, cL:cL + C])
                        p_i = ps_i.tile([H, C], F32, tag="i")
                        p_o = ps_o.tile([H, C], F32, tag="o")
                        p_t = ps_t.tile([H, C], F32, tag="t")
                        for (p, plo, phi) in pieces:
                            base = p * 4 * H
                            nc.tensor.matmul(
                                p_i[:, plo - lo:phi - lo],
                                w_sb[:, base + H:base + 2 * H],
                                ain[:, plo:phi], start=True, stop=True)
                            nc.tensor.matmul(
                                p_o[:, plo - lo:phi - lo],
                                w_sb[:, base + 2 * H:base + 3 * H],
                                ain[:, plo:phi], start=True, stop=True)
                            nc.tensor.matmul(
                                p_t[:, plo - lo:phi - lo],
                                w_sb[:, base + 3 * H:base + 4 * H],
                                ain[:, plo:phi], start=True, stop=True)
                        si_sb = wk.tile([H, C], F16, tag="si")
                        nc.scalar.activation(si_sb[:, :], p_i[:, :], Sig,
                                             bias=bias_sb[:, 0:1])
                        tu_sb = wk.tile([H, C], F16, tag="tu")
                        nc.scalar.activation(tu_sb[:, :], p_t[:, :], Tanh,
                                             bias=bias_sb[:, 2:3])
                        nc.vector.tensor_mul(si_sb[:, :], si_sb[:, :],
                                             tu_sb[:, :])
                        c_new = out[:, M0 + lo:M0 + hi]
                        nc.vector.tensor_add(c_new, si_sb[:, :], cr_sb[:, :])
                        so_sb = wk.tile([H, C], F16, tag="so")
                        nc.scalar.activation(so_sb[:, :], p_o[:, :], Sig,
                                             bias=bias_sb[:, 1:2])
                        th_sb = wk.tile([H, C], F16, tag="th")
                        nc.scalar.activation(th_sb[:, :], c_new, Tanh)
                        nc.vector.tensor_mul(out[:, lo:hi], so_sb[:, :],
                                             th_sb[:, :])
                    else:
                        o0 = M0 + lo
                        nc.vector.tensor_mul(out[:, o0:o0 + C], f_sb[:, :],
                                             ain[:, cL:cL + C])
                # store from the ACT queue (HWDGE): keeps SP = loads only,
                # so the next body's load issues before this body's store
                nc.scalar.dma_start(out=OUT[:, :], in_=out[:, :])

            if reps == 1:
                for j in range(bodies):
                    body(j)
            else:
                with tc.For_i(0, reps, 1) as _i:
                    for j in range(bodies):
                        body(j)
    nc.finalize()
    return nc


_BUILD_CACHE = {}


def _prepare(inputs, reps=1, bodies=1):
    global N, H, N_POS, SH
    N, _, H = np.asarray(inputs["child_h"]).shape
    N_POS = np.asarray(inputs["W_f"]).shape[0] // H
    SH = N // NC
    child_h = np.asarray(inputs["child_h"], np.float32).reshape(N, H)
    child_c = np.asarray(inputs["child_c"], np.float32).reshape(N, H)
    e1 = np.asarray(inputs["extra_input_depth_1"], np.float32)
    e2 = np.asarray(inputs["extra_input_depth_2"], np.float32)
    h_prev = np.asarray(inputs["h_prev"], np.float32)
    pos = np.asarray(inputs["pos"]).astype(np.int64)
    depth = np.asarray(inputs["depth"]).astype(np.int64)
    mask = np.asarray(inputs["mask"]).astype(np.int64)
    W_f = np.asarray(inputs["W_f"], np.float32)
    b_f = np.asarray(inputs["b_f"], np.float32)
    W_iou = np.asarray(inputs["W_iou"], np.float32)
    b_iou = np.asarray(inputs["b_iou"], np.float32)

    mask01 = (mask != 0).astype(np.int64)
    plan = _plan(pos, depth, mask01)
    L, M0, Lin, Lout = plan["L"], plan["M0"], plan["Lin"], plan["Lout"]

    key = (tuple(sorted((k, v) for k, v in plan["blocks"].items())),
           Lin, Lout, reps, bodies)
    if key not in _BUILD_CACHE:
        _BUILD_CACHE[key] = _build(plan, reps=reps, bodies=bodies)
    nc = _BUILD_CACHE[key]

    # weights fp16 [H, 10*4*H + 10*H]: per pos [W_f | WiT | WoT | WuT];
    # then b_f on row 0, per pos at WCOLS + p*H (rank-1 bias matmuls)
    Wp = np.zeros((H, N_POS * 4 * H + N_POS * H), np.float16)
    W_f_r = W_f.reshape(N_POS, H, H)
    b_f_r = b_f.reshape(N_POS, H)
    for p in range(N_POS):
        base = p * 4 * H
        Wp[:, base:base + H] = W_f_r[p]
        for j in range(3):
            Wp[:, base + (j + 1) * H:base + (j + 2) * H] = \
                W_iou[j * H:(j + 1) * H, p * H:(p + 1) * H].T
    Wp[0, N_POS * 4 * H:] = b_f_r.reshape(-1)

    bias = np.empty((H, 3 + N_POS), np.float32)
    bias[:, 0] = b_iou[0, 0:H]
    bias[:, 1] = b_iou[0, H:2 * H]
    bias[:, 2] = b_iou[0, 2 * H:3 * H]
    bias[:, 3:] = b_f.reshape(N_POS, H).T

    OHm = np.ones((1, WIN), np.float16)

    e_src = np.where((depth == 1)[:, None], e1, e2).astype(np.float32)
    srcs = (child_h, child_c, e_src)

    slot_idx = plan["slot_idx"]
    ain_slot, ain_kind = plan["ain_slot"], plan["ain_kind"]
    out_slot, out_kind = plan["out_slot"], plan["out_kind"]

    in_maps = []
    for c in range(NC):
        node = np.where(ain_slot >= 0, slot_idx[c][ain_slot], -1)
        AINm = np.zeros((H, Lin), np.float16)
        for kind in range(3):
            mm = (ain_kind == kind) & (node >= 0)
            AINm[:, mm] = srcs[kind][node[mm]].T.astype(np.float16)
        in_maps.append({"AIN": AINm, "W": Wp, "OH": OHm, "BIAS": bias})

    mask_on = mask != 0

    def assemble(results):
        h = np.empty((N, H), np.float32)
        cc = np.empty((N, H), np.float32)
        for c in range(NC):
            node = slot_idx[c][out_slot]
            O = results[c]["OUT"]
            mh = (out_kind == 0) & (node >= 0)
            h[node[mh]] = O[:, mh].T.astype(np.float32)
            mc = (out_kind != 0) & (node >= 0)
            cc[node[mc]] = O[:, mc].T.astype(np.float32)
        h[mask_on] = h_prev[mask_on]
        return h, cc

    return nc, in_maps, assemble


def kernel(**inputs):
    nc, in_maps, assemble = _prepare(inputs)
    try:
        res = run_bass_kernel_spmd(nc, in_maps, list(range(NC)))
    except Exception:
        # first execution of a freshly compiled NEFF occasionally kills the
        # worker (transient); one retry has always succeeded
        res = run_bass_kernel_spmd(nc, in_maps, list(range(NC)))
    LAST["results"] = res
    LAST["nc"] = nc
    return assemble(res.results)
